# revision 15
# baseline (speedup 1.0000x reference)
"""AttentiveStatsPooling Trainium2 kernel.

Full-input contract: kernel(**inputs) takes the unsharded numpy inputs
  x            (32, 1536, 2048) f32
  padding_mask (32, 2048)       bool
  W_tdnn       (128, 1536)      f32
  b_tdnn       (128,)           f32
  W_attn       (1536, 128)      f32
  b_attn       (1536,)          f32
and returns the full (32, 3072) f32 output.

Sharding: data-parallel over batch. 8 cores x 4 samples each, weights
replicated. Math per sample:
  e    = tanh(W_tdnn @ x + b_tdnn)            (BN, T)
  a    = W_attn @ e  (+ b_attn: dropped - constant along T, cancels in
                      the softmax over T)      (C, T)
  a   += -1e9 * mask[t]                        (additive mask; exp -> 0)
  S0   = sum_t exp(a);  S1 = sum_t exp(a)*x;  S2 = sum_t exp(a)*x^2
  mean = S1/S0;  std = sqrt(clip(S2/S0 - mean^2, 1e-9))
All matmuls/products in bf16 with fp32 accumulation (PSUM / reduce
accumulators): HW-verified relative error 2.1e-4 (scale-rel absmax 7.5e-4).

Performance (measured on HW, ~300-330 us/core; 1.7x over the naive
schedule). Engine assignment chosen from on-HW microbenchmarks:
  - any DVE op with accum_out runs 1x (fast perf-modes disabled by the
    accumulator), so reductions cost ~2.2us/[128,2048] everywhere;
  - products (tensor_tensor bf16) do hit the 2x mode (1.17us);
  - exp on ACT reads logits straight from PSUM, its accumulator gives
    S0 for free; S1 reduces on DVE, S2 on ACT (Copy+accum) to balance
    both engines at ~235us busy;
  - the S2 stage is pipelined 2 steps behind, and two samples' chunk
    streams are interleaved so each engine fills the other stream's
    cross-engine dependency bubbles (the single biggest win).
"""

import numpy as np
import ml_dtypes

B, C, T = 32, 1536, 2048
BN = 128
NCORES = 8
SPC = B // NCORES  # samples per core
CK = C // 128      # c chunks of 128 partitions
NJ = T // 512      # 512-wide column groups (one PSUM bank each)

BF16 = ml_dtypes.bfloat16

_PROG_CACHE = {}


def _build_program(reps=None):
    """Build the per-core program. reps=None: straight-line body.
    reps=K: wrap the whole body in a hardware For_i loop (timing only)."""
    import concourse.bacc as bacc
    import concourse.tile as tile
    import concourse.mybir as mybir
    from contextlib import nullcontext
    from concourse.bass_interp import get_hw_module

    dt = mybir.dt
    AF = mybir.ActivationFunctionType
    OP = mybir.AluOpType

    nc = bacc.Bacc(
        "TRN2",
        target_bir_lowering=False,
        debug=False,
        num_devices=NCORES,
        num_swdge_queues=4,
    )
    x_d = nc.dram_tensor("x", [SPC, C, T], dt.bfloat16, kind="ExternalInput")
    mn_d = nc.dram_tensor("maskneg", [SPC, T], dt.bfloat16, kind="ExternalInput")
    wt_d = nc.dram_tensor("wt", [C, BN], dt.bfloat16, kind="ExternalInput")
    wa_d = nc.dram_tensor("wa", [BN, C], dt.bfloat16, kind="ExternalInput")
    bt_d = nc.dram_tensor("bt", [BN, 1], dt.float32, kind="ExternalInput")
    out_d = nc.dram_tensor("out", [SPC, 2 * C], dt.float32, kind="ExternalOutput")

    with tile.TileContext(nc) as tc:
        with (
            tc.tile_pool(name="const", bufs=1) as constp,
            tc.tile_pool(name="xin", bufs=2 * CK) as xp,
            tc.tile_pool(name="esb", bufs=3) as ep,
            tc.tile_pool(name="expm", bufs=3) as xpm,
            tc.tile_pool(name="prod", bufs=4) as prp,
            tc.tile_pool(name="mneg", bufs=2) as mnp,
            tc.tile_pool(name="s0p", bufs=4) as s0pp,
            tc.tile_pool(name="stats", bufs=1) as statsp,
            tc.tile_pool(name="tail", bufs=2) as tailp,
            tc.tile_pool(name="ps", bufs=2, space="PSUM") as psp,
        ):
            # ---- constants ------------------------------------------------
            wt_sb = constp.tile([128, CK, BN], dt.bfloat16, tag="wt")
            nc.sync.dma_start(
                out=wt_sb, in_=wt_d.ap().rearrange("(k p) o -> p k o", p=128)
            )
            wa_sb = constp.tile([128, C], dt.bfloat16, tag="wa")
            nc.sync.dma_start(out=wa_sb, in_=wa_d.ap())
            bt_sb = constp.tile([128, 1], dt.float32, tag="bt")
            nc.sync.dma_start(out=bt_sb, in_=bt_d.ap())
            ones_sb = constp.tile([1, 128], dt.bfloat16, tag="ones")
            nc.vector.memset(ones_sb, 1.0)

            loop_cm = tc.For_i(0, reps, 1) if reps is not None else nullcontext()
            with loop_cm:
                stats = []
                for s in range(SPC):
                    S0 = statsp.tile([128, CK], dt.float32, tag=f"S0_{s}")
                    S1 = statsp.tile([128, CK], dt.float32, tag=f"S1_{s}")
                    S2 = statsp.tile([128, CK], dt.float32, tag=f"S2_{s}")
                    stats.append((S0, S1, S2))

                # process samples in pairs; the two chunk streams interleave
                # so ACT/DVE always have an independent chunk to work on
                for s0 in range(0, SPC, 2):
                    pair = [s0, s0 + 1]
                    xts = {}
                    mnegs = {}
                    esbs = {}
                    for s in pair:
                        mneg_sb = mnp.tile(
                            [1, T], dt.bfloat16, tag="mneg", name=f"mneg_{s}"
                        )
                        nc.sync.dma_start(out=mneg_sb, in_=mn_d.ap()[s : s + 1, :])
                        mnegs[s] = mneg_sb
                        for k in range(CK):
                            xt = xp.tile(
                                [128, T], dt.bfloat16, tag="x", name=f"x_{s}_{k}"
                            )
                            nc.sync.dma_start(
                                out=xt, in_=x_d.ap()[s, k * 128 : (k + 1) * 128, :]
                            )
                            xts[(s, k)] = xt

                    # mm1 + tanh for both samples of the pair
                    for s in pair:
                        pse = psp.tile(
                            [128, T], dt.float32, tag="ps", name=f"pse_{s}"
                        )
                        for j in range(NJ):
                            for k in range(CK):
                                nc.tensor.matmul(
                                    pse[:, j * 512 : (j + 1) * 512],
                                    lhsT=wt_sb[:, k, :],
                                    rhs=xts[(s, k)][:, j * 512 : (j + 1) * 512],
                                    start=(k == 0),
                                    stop=(k == CK - 1),
                                )
                        e_sb = ep.tile([128, T], dt.bfloat16, tag="e", name=f"e_{s}")
                        nc.scalar.activation(
                            out=e_sb, in_=pse, func=AF.Tanh, bias=bt_sb, scale=1.0
                        )
                        esbs[s] = e_sb

                    def s2_stage(s, c, p2):
                        if (s * CK + c) % 16 == 0:
                            nc.vector.tensor_reduce(
                                out=stats[s][2][:, c : c + 1],
                                in_=p2,
                                op=OP.add,
                                axis=mybir.AxisListType.X,
                            )
                        else:
                            junk = prp.tile(
                                [128, T], dt.bfloat16, tag="junk",
                                name=f"junk_{s}_{c}",
                            )
                            nc.scalar.activation(
                                out=junk,
                                in_=p2,
                                func=AF.Copy,
                                accum_out=stats[s][2][:, c : c + 1],
                            )

                    pending = []
                    for c in range(CK):
                        for s in pair:
                            S0, S1, S2 = stats[s]
                            e_sb = esbs[s]
                            mneg_sb = mnegs[s]
                            expm = xpm.tile(
                                [128, T], dt.bfloat16, tag="expm",
                                name=f"expm_{s}_{c}",
                            )
                            pa = psp.tile(
                                [128, T], dt.float32, tag="ps", name=f"pa_{s}_{c}"
                            )
                            for jj in range(NJ):
                                nc.tensor.matmul(
                                    pa[:, jj * 512 : (jj + 1) * 512],
                                    lhsT=wa_sb[:, c * 128 : (c + 1) * 128],
                                    rhs=e_sb[:, jj * 512 : (jj + 1) * 512],
                                    start=True,
                                    stop=False,
                                )
                            for jj in range(NJ):
                                nc.tensor.matmul(
                                    pa[:, jj * 512 : (jj + 1) * 512],
                                    lhsT=ones_sb[:, :],
                                    rhs=mneg_sb[:, jj * 512 : (jj + 1) * 512],
                                    start=False,
                                    stop=True,
                                )
                            nc.scalar.activation(
                                out=expm,
                                in_=pa,
                                func=AF.Exp,
                                accum_out=S0[:, c : c + 1],
                            )
                            p1 = prp.tile(
                                [128, T], dt.bfloat16, tag="p1", name=f"p1_{s}_{c}"
                            )
                            nc.vector.tensor_tensor(
                                out=p1, in0=expm, in1=xts[(s, c)], op=OP.mult
                            )
                            p2 = prp.tile(
                                [128, T], dt.bfloat16, tag="p2", name=f"p2_{s}_{c}"
                            )
                            nc.vector.tensor_tensor(
                                out=p2, in0=p1, in1=xts[(s, c)], op=OP.mult
                            )
                            nc.vector.tensor_reduce(
                                out=S1[:, c : c + 1],
                                in_=p1,
                                op=OP.add,
                                axis=mybir.AxisListType.X,
                            )
                            pending.append((s, c, p2))
                            if len(pending) > 2:
                                s2_stage(*pending.pop(0))
                    for item in pending:
                        s2_stage(*item)

                # ---- tail: mean/std + output DMA --------------------------
                for s in range(SPC):
                    S0, S1, S2 = stats[s]
                    r0 = tailp.tile([128, CK], dt.float32, tag="r0", name=f"r0_{s}")
                    nc.vector.reciprocal(out=r0, in_=S0)
                    mean = tailp.tile(
                        [128, CK], dt.float32, tag="mean", name=f"mean_{s}"
                    )
                    nc.vector.tensor_tensor(out=mean, in0=S1, in1=r0, op=OP.mult)
                    ex2 = tailp.tile([128, CK], dt.float32, tag="ex2", name=f"ex2_{s}")
                    nc.vector.tensor_tensor(out=ex2, in0=S2, in1=r0, op=OP.mult)
                    m2 = tailp.tile([128, CK], dt.float32, tag="m2", name=f"m2_{s}")
                    nc.vector.tensor_tensor(out=m2, in0=mean, in1=mean, op=OP.mult)
                    var = tailp.tile([128, CK], dt.float32, tag="var", name=f"var_{s}")
                    nc.vector.tensor_tensor(out=var, in0=ex2, in1=m2, op=OP.subtract)
                    nc.vector.tensor_scalar(
                        out=var,
                        in0=var,
                        scalar1=1e-9,
                        scalar2=None,
                        op0=OP.max,
                    )
                    std = tailp.tile([128, CK], dt.float32, tag="std", name=f"std_{s}")
                    nc.scalar.activation(out=std, in_=var, func=AF.Sqrt)
                    nc.sync.dma_start(
                        out=out_d.ap()[s, 0:C].rearrange("(ck p) -> p ck", p=128),
                        in_=mean,
                    )
                    nc.sync.dma_start(
                        out=out_d.ap()[s, C : 2 * C].rearrange(
                            "(ck p) -> p ck", p=128
                        ),
                        in_=std,
                    )

    nc.compile()
    nc.m = get_hw_module(nc.m)
    return nc


def _get_program():
    if "nc" not in _PROG_CACHE:
        _PROG_CACHE["nc"] = _build_program()
    return _PROG_CACHE["nc"]


def _prep_inputs(x, padding_mask, W_tdnn, b_tdnn, W_attn, b_attn):
    """Host-side prep: cast/transpose, build per-core input maps."""
    xb = np.ascontiguousarray(x).astype(BF16)
    maskneg = np.where(padding_mask, np.float32(-1e9), np.float32(0.0)).astype(BF16)
    wt = np.ascontiguousarray(W_tdnn.T).astype(BF16)  # (C, BN)
    wa = np.ascontiguousarray(W_attn.T).astype(BF16)  # (BN, C)
    bt = np.ascontiguousarray(b_tdnn.astype(np.float32).reshape(BN, 1))
    in_maps = []
    for i in range(NCORES):
        sl = slice(i * SPC, (i + 1) * SPC)
        in_maps.append(
            {
                "x": np.ascontiguousarray(xb[sl]),
                "maskneg": np.ascontiguousarray(maskneg[sl]),
                "wt": wt,
                "wa": wa,
                "bt": bt,
            }
        )
    return in_maps


def kernel(x, padding_mask, W_tdnn, b_tdnn, W_attn, b_attn):
    from concourse.bass_utils import run_bass_kernel_spmd

    nc = _get_program()
    in_maps = _prep_inputs(x, padding_mask, W_tdnn, b_tdnn, W_attn, b_attn)
    res = run_bass_kernel_spmd(nc, in_maps, core_ids=list(range(NCORES)))
    out = np.concatenate([res.results[i]["out"] for i in range(NCORES)], axis=0)
    return out.astype(np.float32)
